# revision 3
# baseline (speedup 1.0000x reference)
"""Trainium2 Bass kernel v5 for nn_EquivariantGNN_GAT (2-layer GAT + linear + mean pool).

The edge phase is SWDGE descriptor-generation bound (~8ns per gathered row,
intrinsic — measured contention-free).  v4 keeps the per-edge pair-gather and
makes every consumer cheap and group-granular so the gather engine never
stalls:

  - Degree-sorted relabeling + interleaved block->core assignment: dst blocks
    have near-uniform in-degree; edges are dst-major (partition = dst node,
    free axis = edge rank), so per-dst softmax needs no one-hot expansion.
  - int16 gather indices address node PAIRS (elem 1024B = 2 rows, idx=row>>1),
    keeping indices < 32768.  A host mask (interleaved even/odd per chunk)
    sends wrong-parity and pad slots to exp(-inf)=0.
  - Per gather group (<=8 chunks): scores Q/V/exp on [P, 2sn] views, then ONE
    tensor_tensor multiply Gt*Pe into a [P, 129, sn, 2] scaled tile and ONE
    tensor_reduce -> [P, 129] partials chained into an SBUF f32 accumulator
    (odd parity), while even-parity rows go through per-chunk diagonal
    matmuls (MT = ident*Pe) accumulated in PSUM.  Gathers round-robin over
    two SWDGE queues (overlaps descriptor generation with ring drain: the
    single-queue rate of ~8.4ns/row improves to ~5.7ns/row).
  - Self loops come from the SBUF-resident own-shard rows (no DMA).
  - emb[z] lookup via host-built one-hot matmul (keeps gpsimd gather-only).

kernel(**inputs) takes FULL problem inputs, returns the [64, 32] output.
"""
import sys

sys.path.insert(0, "/opt/trn_rl_repo")

import ml_dtypes
import numpy as np

import concourse.bass as bass
import concourse.bacc as bacc
import concourse.mybir as mybir
import concourse.tile as tile
import concourse.bass_utils as bass_utils
from concourse.bass_interp import get_hw_module

N = 50000
E = 1600000
H = 128
O = 32
T = 100
G = 64
P = 128
NCORES = 8
NBPC = 49
NB = NBPC * NCORES
NPAD = NB * P
SH = NBPC * P
ROW = 256              # bf16 elems per hs row
RW = 2 * ROW           # gathered pair: two rows
WCOL = 131
GMAX = 8               # chunks per dma_gather
NEG = 0.2
NPADN = NPAD - N       # 176 pad nodes (lowest ids after relabel)

F32 = mybir.dt.float32
BF16 = mybir.dt.bfloat16
I16 = mybir.dt.int16
ALU = mybir.AluOpType
AF = mybir.ActivationFunctionType
AX = mybir.AxisListType
NPBF = ml_dtypes.bfloat16


def _wrap16(flat):
    n = flat.shape[0]
    assert n % 16 == 0
    w = flat.reshape(n // 16, 16).T
    return np.tile(w, (8, 1))


# ---------------------------------------------------------------- host prep
def _prep(inputs):
    pos = np.asarray(inputs["pos"], np.float32)
    z = np.asarray(inputs["z"]).astype(np.int64)
    ei = np.asarray(inputs["edge_index"]).astype(np.int64)
    batch = np.asarray(inputs["batch"]).astype(np.int64)

    src_o, dst_o = ei[0], ei[1]
    indeg = np.bincount(dst_o, minlength=N)

    # relabel: pad nodes first, then real nodes ascending by in-degree
    order = np.argsort(indeg, kind="stable")
    newid = np.empty(N, np.int64)
    newid[order] = NPADN + np.arange(N)
    old_of = np.full(NPAD, -1, np.int64)
    old_of[NPADN + np.arange(N)] = order

    # block of new node n: g = n//P -> core g%8, slot g//8
    gs = np.arange(NPAD) // P
    tbl = (gs % NCORES) * SH + (gs // NCORES) * P + (np.arange(NPAD) % P)

    sn_ = newid[src_o]
    dn_ = newid[dst_o]
    deg = np.zeros(NPAD, np.int64)
    deg[NPADN:] = indeg[order]

    CH = np.zeros(NBPC, np.int64)
    for i in range(NBPC):
        CH[i] = max(1, int(deg[min((8 * i + 8) * P, NPAD) - 1]))
    off = np.zeros(NBPC + 1, np.int64)
    off[1:] = np.cumsum(CH)
    CHT = int(off[-1])

    eorder = np.argsort(dn_, kind="stable")
    ds = dn_[eorder]
    ss = sn_[eorder]
    starts = np.searchsorted(ds, np.arange(NPAD))
    rank = np.arange(E) - starts[ds]

    core_e = (ds // P) % NCORES
    slot_e = (ds // P) // NCORES
    part_e = ds % P
    colg_e = off[slot_e] + rank

    tgt = tbl[ss]
    idx_all = np.zeros((NCORES, P, CHT), np.int64)
    # interleaved even/odd mask: col 2k+parity; -1e30 = dead slot
    mEO = np.full((NCORES, P, 2 * CHT), -1e30, np.float32)
    idx_all[core_e, part_e, colg_e] = tgt >> 1
    mEO[core_e, part_e, 2 * colg_e + (tgt % 2)] = 0.0

    idx_w = np.zeros((NCORES, P, 8 * CHT), np.int16)
    for c in range(NCORES):
        for i in range(NBPC):
            fl = idx_all[c, :, off[i]:off[i + 1]].T.reshape(-1)  # (k*P+p)
            idx_w[c, :, 8 * off[i]:8 * off[i + 1]] = _wrap16(
                fl.astype(np.int16))

    batchrel = np.full((NCORES, P, NBPC), -1.0, np.float32)
    pos_shard = np.zeros((NCORES, SH, 3), np.float32)
    zoht = np.zeros((NCORES, P, SH), NPBF)
    for c in range(NCORES):
        gsl = 8 * np.arange(NBPC) + c
        nid = (gsl[:, None] * P + np.arange(P)[None, :]).reshape(-1)
        oldid = old_of[nid]
        valid = oldid >= 0
        br = np.full(SH, -1.0, np.float32)
        br[valid] = batch[oldid[valid]].astype(np.float32)
        batchrel[c] = br.reshape(NBPC, P).T
        pos_shard[c][valid] = pos[oldid[valid]]
        zz = z[oldid[valid]].astype(np.int64)
        cols = np.nonzero(valid)[0]
        zoht[c][zz, cols] = 1.0

    counts = np.bincount(batch, minlength=G).astype(np.float32)
    cinv = (1.0 / np.maximum(counts, 1.0)).astype(np.float32).reshape(G, 1)

    iota_f = np.tile(np.arange(P, dtype=np.float32), (P, 1))
    emb_pad = np.zeros((P, 125), NPBF)
    emb_pad[:T] = np.asarray(inputs["emb"], np.float32).astype(NPBF)
    identrep = np.tile(np.eye(P, dtype=NPBF), (1, GMAX))

    consts = dict(
        iota=np.ascontiguousarray(iota_f),
        ident=np.eye(P, dtype=np.float32),
        identrep=np.ascontiguousarray(identrep),
        emb=np.ascontiguousarray(emb_pad),
        W1=np.ascontiguousarray(np.asarray(inputs["W1"], np.float32)),
        W1T=np.ascontiguousarray(np.asarray(inputs["W1"], np.float32).T),
        a1s=np.asarray(inputs["a1_src"], np.float32).reshape(H, 1),
        a1d=np.asarray(inputs["a1_dst"], np.float32).reshape(H, 1),
        b1rep=np.ascontiguousarray(
            np.tile(np.asarray(inputs["b1"], np.float32), (P, 1))),
        W2=np.ascontiguousarray(np.asarray(inputs["W2"], np.float32)),
        W2T=np.ascontiguousarray(np.asarray(inputs["W2"], np.float32).T),
        a2s=np.asarray(inputs["a2_src"], np.float32).reshape(H, 1),
        a2d=np.asarray(inputs["a2_dst"], np.float32).reshape(H, 1),
        b2rep=np.ascontiguousarray(
            np.tile(np.asarray(inputs["b2"], np.float32), (P, 1))),
        Wlin=np.ascontiguousarray(np.asarray(inputs["Wlin"], np.float32)),
        blinrep=np.ascontiguousarray(
            np.tile(np.asarray(inputs["blin"], np.float32), (P, 1))),
        cinv=cinv,
    )

    meta = dict(CH=tuple(int(x) for x in CH),
                off=tuple(int(x) for x in off), CHT=CHT)
    percore = dict(idx_w=idx_w, batchrel=batchrel, pos_shard=pos_shard,
                   zoht=zoht, maskEO=mEO.astype(NPBF))
    return meta, percore, consts


# ---------------------------------------------------------------- program
def _build(meta, analysis=False):
    CH = meta["CH"]
    off = meta["off"]
    CHT = meta["CHT"]

    nc = bacc.Bacc("TRN2", target_bir_lowering=False, debug=False,
                   enable_asserts=False,
                   num_devices=1 if analysis else NCORES,
                   num_swdge_queues=2,
                   dynamic_dma_scratch_size=16384)

    t_idx = nc.dram_tensor("idx_w", [P, 8 * CHT], I16, kind="ExternalInput")
    t_mEO = nc.dram_tensor("maskEO", [P, 2 * CHT], BF16, kind="ExternalInput")
    t_brel = nc.dram_tensor("batchrel", [P, NBPC], F32, kind="ExternalInput")
    t_pos = nc.dram_tensor("pos_shard", [SH, 3], F32, kind="ExternalInput")
    t_zoht = nc.dram_tensor("zoht", [P, SH], BF16, kind="ExternalInput")
    t_emb = nc.dram_tensor("emb", [P, 125], BF16, kind="ExternalInput")
    t_identrep = nc.dram_tensor("identrep", [P, GMAX * P], BF16,
                                kind="ExternalInput")
    t_iota = nc.dram_tensor("iota", [P, P], F32, kind="ExternalInput")
    t_ident = nc.dram_tensor("ident", [P, P], F32, kind="ExternalInput")
    t_W1 = nc.dram_tensor("W1", [H, H], F32, kind="ExternalInput")
    t_W1T = nc.dram_tensor("W1T", [H, H], F32, kind="ExternalInput")
    t_a1s = nc.dram_tensor("a1s", [H, 1], F32, kind="ExternalInput")
    t_a1d = nc.dram_tensor("a1d", [H, 1], F32, kind="ExternalInput")
    t_b1rep = nc.dram_tensor("b1rep", [P, H], F32, kind="ExternalInput")
    t_W2 = nc.dram_tensor("W2", [H, H], F32, kind="ExternalInput")
    t_W2T = nc.dram_tensor("W2T", [H, H], F32, kind="ExternalInput")
    t_a2s = nc.dram_tensor("a2s", [H, 1], F32, kind="ExternalInput")
    t_a2d = nc.dram_tensor("a2d", [H, 1], F32, kind="ExternalInput")
    t_b2rep = nc.dram_tensor("b2rep", [P, H], F32, kind="ExternalInput")
    t_Wlin = nc.dram_tensor("Wlin", [H, O], F32, kind="ExternalInput")
    t_blinrep = nc.dram_tensor("blinrep", [P, O], F32, kind="ExternalInput")
    t_cinv = nc.dram_tensor("cinv", [G, 1], F32, kind="ExternalInput")
    t_out = nc.dram_tensor("out", [G, O], F32, kind="ExternalOutput")

    groups = [list(range(NCORES))]

    with tile.TileContext(nc) as tc:
        with (
            tc.tile_pool(name="const", bufs=1) as cpool,
            tc.tile_pool(name="work", bufs=3) as wpool,
            tc.tile_pool(name="gat", bufs=6) as gpool,
            tc.tile_pool(name="sc", bufs=3) as scpool,
            tc.tile_pool(name="sml", bufs=6) as spool,
            tc.tile_pool(name="ps", bufs=2, space="PSUM") as pspool,
            tc.tile_pool(name="acc", bufs=1, space="PSUM") as apool,
            tc.tile_pool(name="dram", bufs=1, space="DRAM") as dpool,
        ):
            def cload(nm, t, shape, dtype=F32):
                tl = cpool.tile(shape, dtype, name=nm, tag=nm)
                nc.sync.dma_start(tl[:, :], t.ap())
                return tl

            iota_sb = cload("iota_sb", t_iota, [P, P])
            ident_sb = cload("ident_sb", t_ident, [P, P])
            b1rep_sb = cload("b1rep_sb", t_b1rep, [P, H])
            b2rep_sb = cload("b2rep_sb", t_b2rep, [P, H])
            Wlin_sb = cload("Wlin_sb", t_Wlin, [H, O])
            blinrep_sb = cload("blinrep_sb", t_blinrep, [P, O])
            cinv_sb = cload("cinv_sb", t_cinv, [G, 1])
            idx_sb = cload("idx_sb", t_idx, [P, 8 * CHT], I16)
            mEO_sb = cload("mEO_sb", t_mEO, [P, 2 * CHT], BF16)
            brel_sb = cload("brel_sb", t_brel, [P, NBPC])
            zoht_sb = cload("zoht_sb", t_zoht, [P, SH], BF16)
            emb_sb = cload("emb_sb", t_emb, [P, 125], BF16)
            identrep_sb = cload("identrep_sb", t_identrep,
                                [P, GMAX * P], BF16)
            zeros64 = cpool.tile([P, G], F32, name="zeros64", tag="zeros64")
            nc.vector.memset(zeros64[:, :], 0.0)
            zeros129 = cpool.tile([P, 129], F32, name="zeros129",
                                  tag="zeros129")
            nc.vector.memset(zeros129[:, :], 0.0)

            def fold(nm, tW, tWT, tas, tad):
                We = cpool.tile([H, WCOL], F32, name=nm, tag=nm)
                nc.sync.dma_start(We[:, 0:H], tW.ap())
                nc.vector.memset(We[:, 128:129], 0.0)
                WT_sb = cpool.tile([H, H], F32, name=nm + "_WT", tag=nm + "_WT")
                nc.sync.dma_start(WT_sb[:, :], tWT.ap())
                for col, tvec in ((129, tas), (130, tad)):
                    av = cpool.tile([H, 1], F32, name=f"{nm}_a{col}",
                                    tag=f"{nm}_a{col}")
                    nc.sync.dma_start(av[:, :], tvec.ap())
                    fps = pspool.tile([H, 1], F32, name=f"{nm}_f{col}",
                                      tag="tps")
                    nc.tensor.matmul(fps[:, :], lhsT=WT_sb[:, :], rhs=av[:, :],
                                     start=True, stop=True)
                    nc.scalar.activation(We[:, col:col + 1], fps[:, :],
                                         AF.Copy)
                return We

            W1e = fold("W1e", t_W1, t_W1T, t_a1s, t_a1d)
            W2e = fold("W2e", t_W2, t_W2T, t_a2s, t_a2d)

            hs1_own = cpool.tile([P, NBPC * ROW], BF16, name="hs1_own",
                                 tag="hs1_own")
            hs2_own = cpool.tile([P, NBPC * ROW], BF16, name="hs2_own",
                                 tag="hs2_own")
            s1src = cpool.tile([P, NBPC], F32, name="s1src", tag="s1src")
            s1dst = cpool.tile([P, NBPC], F32, name="s1dst", tag="s1dst")
            s2src = cpool.tile([P, NBPC], F32, name="s2src", tag="s2src")
            s2dst = cpool.tile([P, NBPC], F32, name="s2dst", tag="s2dst")

            hs1_sh = dpool.tile([SH, ROW], BF16, name="hs1_sh", tag="hs1_sh")
            hs1_f = dpool.tile([NPAD, ROW], BF16, name="hs1_f", tag="hs1_f",
                               addr_space="Shared")
            hs2_sh = dpool.tile([SH, ROW], BF16, name="hs2_sh", tag="hs2_sh")
            hs2_f = dpool.tile([NPAD, ROW], BF16, name="hs2_f", tag="hs2_f",
                               addr_space="Shared")
            pool_in = dpool.tile([G, O], F32, name="pool_in", tag="pool_in")
            pool_out = dpool.tile([G, O], F32, name="pool_out",
                                  tag="pool_out", addr_space="Shared")

            def store_hs(i, hs_ps, own, shd, ssrc_all, sdst_all):
                nc.scalar.activation(own[:, i * ROW:i * ROW + WCOL],
                                     hs_ps[:, :], AF.Copy)
                nc.vector.memset(own[:, i * ROW + 128:i * ROW + 129], 1.0)
                nc.vector.memset(own[:, i * ROW + WCOL:(i + 1) * ROW], 0.0)
                nc.scalar.activation(ssrc_all[:, i:i + 1],
                                     hs_ps[:, 129:130], AF.Copy)
                nc.scalar.activation(sdst_all[:, i:i + 1],
                                     hs_ps[:, 130:131], AF.Copy)
                nc.sync.dma_start(shd[i * P:(i + 1) * P, :],
                                  own[:, i * ROW:(i + 1) * ROW])

            # ---------------- stage A ----------------
            for i in range(NBPC):
                e_ps = pspool.tile([P, 125], F32, name="e_ps", tag="tps")
                nc.tensor.matmul(e_ps[:, :],
                                 lhsT=zoht_sb[:, i * P:(i + 1) * P],
                                 rhs=emb_sb[:, :], start=True, stop=True)
                x1 = wpool.tile([P, H], F32, name="x1", tag="x1")
                nc.sync.dma_start(x1[:, 0:3], t_pos.ap()[i * P:(i + 1) * P, :])
                nc.scalar.activation(x1[:, 3:128], e_ps[:, :], AF.Copy)
                xt_ps = pspool.tile([P, P], F32, name="xt_ps", tag="tps")
                nc.tensor.transpose(xt_ps[:, :], x1[:, :], ident_sb[:, :])
                x1t = wpool.tile([P, P], F32, name="x1t", tag="x1t")
                nc.scalar.activation(x1t[:, :], xt_ps[:, :], AF.Copy)
                hs_ps = pspool.tile([P, WCOL], F32, name="hs_ps", tag="hsps")
                nc.tensor.matmul(hs_ps[:, :], lhsT=x1t[:, :], rhs=W1e[:, :],
                                 start=True, stop=True)
                store_hs(i, hs_ps, hs1_own, hs1_sh, s1src, s1dst)

            if analysis:
                nc.sync.dma_start(hs1_f[0:SH, :], hs1_sh[:, :])
            else:
                nc.gpsimd.collective_compute(
                    "AllGather", ALU.bypass, groups,
                    ins=[hs1_sh[:, :]], outs=[hs1_f[:, :]])

            # ---------------- edge phase ----------------
            def edge_phase(hs_f, own, ssrc_all, sdst_all, post_block):
                table = hs_f[:, :].rearrange("(n two) s -> n (two s)", two=2)
                for i in range(NBPC):
                    nch = CH[i]
                    sdst_col = sdst_all[:, i:i + 1]
                    # self loop from own shard: acc = exp(lrelu(qs)) * own
                    qs = spool.tile([P, 1], F32, name="qs", tag="qs")
                    nc.vector.tensor_scalar(
                        out=qs[:, :], in0=ssrc_all[:, i:i + 1],
                        scalar1=sdst_col, scalar2=None, op0=ALU.add)
                    vs = spool.tile([P, 1], F32, name="vs", tag="vs")
                    nc.vector.scalar_tensor_tensor(
                        out=vs[:, :], in0=qs[:, :], scalar=NEG, in1=qs[:, :],
                        op0=ALU.mult, op1=ALU.max)
                    pes = spool.tile([P, 1], F32, name="pes", tag="pes")
                    nc.scalar.activation(pes[:, :], vs[:, :], AF.Exp)
                    atag = 0
                    acc = spool.tile([P, 129], F32, name="acc0", tag="acc0")
                    nc.vector.scalar_tensor_tensor(
                        out=acc[:, :], in0=own[:, i * ROW:i * ROW + 129],
                        scalar=pes[:, :], in1=zeros129[:, :],
                        op0=ALU.mult, op1=ALU.add)
                    num_ps = pspool.tile([P, 129], F32, name="num_ps",
                                         tag="numps")

                    work = []
                    for c0 in range(0, nch, GMAX):
                        sn = min(GMAX, nch - c0)
                        cb = off[i] + c0
                        Gt = gpool.tile([P, sn * RW], BF16, name="Gt",
                                        tag="Gt")
                        nc.gpsimd.dma_gather(
                            out_ap=Gt.rearrange("p (c s) -> p c s", s=RW),
                            in_ap=table,
                            idxs_ap=idx_sb[:, 8 * cb:8 * (cb + sn)],
                            num_idxs=sn * P,
                            num_idxs_reg=sn * P,
                            elem_size=RW,
                            queue_num=(c0 // GMAX) % 2,
                        )
                        work.append((Gt, sn, cb))

                    nmm = 0
                    for Gt, sn, cb in work:
                        # ssrc for all 2sn (chunk,parity) slots: offset 129
                        G4 = Gt.rearrange("p (c t s) -> p c t s", t=2, s=ROW)
                        sv = G4[:, :, :, 129:130].squeeze(3)      # [P,sn,2]
                        mk = mEO_sb[:, 2 * cb:2 * (cb + sn)].rearrange(
                            "p (c t) -> p c t", t=2)
                        Q = spool.tile([P, 2 * sn], F32, name="Q", tag="Q")
                        nc.vector.scalar_tensor_tensor(
                            out=Q.rearrange("p (c t) -> p c t", t=2),
                            in0=sv, scalar=sdst_col, in1=mk,
                            op0=ALU.add, op1=ALU.add)
                        V = spool.tile([P, 2 * sn], F32, name="V", tag="V")
                        nc.vector.scalar_tensor_tensor(
                            out=V[:, :], in0=Q[:, :], scalar=NEG,
                            in1=Q[:, :], op0=ALU.mult, op1=ALU.max)
                        Pe = spool.tile([P, 2 * sn], F32, name="Pe", tag="Pe")
                        nc.scalar.activation(Pe[:, :], V[:, :], AF.Exp)
                        # even-parity rows: diag-matmul on PE
                        MT = scpool.tile([P, sn * P], BF16, name="MT",
                                         tag="MT")
                        nc.vector.tensor_tensor(
                            out=MT.rearrange("p (c q) -> p c q", q=P),
                            in0=identrep_sb[:, 0:sn * P].rearrange(
                                "p (c q) -> p c q", q=P),
                            in1=Pe.rearrange("p (c t) -> p c t", t=2)
                              [:, :, 0:1].broadcast_to([P, sn, P]),
                            op=ALU.mult)
                        for jj in range(sn):
                            nc.tensor.matmul(
                                num_ps[:, :],
                                lhsT=MT[:, jj * P:(jj + 1) * P],
                                rhs=Gt[:, jj * RW:jj * RW + 129],
                                start=(nmm == 0),
                                stop=(nmm == nch - 1))
                            nmm += 1
                        # odd-parity rows: scale+reduce on DVE
                        Sc = scpool.tile([P, 129 * sn], F32, name="Sc",
                                         tag="Sc")
                        nc.vector.tensor_tensor(
                            out=Sc.rearrange("p (j c) -> p j c", j=129),
                            in0=G4[:, :, 1:2, 0:129].squeeze(2).transpose([0, 2, 1]),
                            in1=Pe.rearrange("p (c t) -> p c t", t=2)
                              [:, :, 1:2].transpose([0, 2, 1])
                              .broadcast_to([P, 129, sn]),
                            op=ALU.mult)
                        red = spool.tile([P, 129], F32, name="red", tag="red")
                        nc.vector.tensor_reduce(
                            out=red[:, :],
                            in_=Sc.rearrange("p (j c) -> p j c", j=129),
                            axis=AX.X, op=ALU.add)
                        atag ^= 1
                        accN = spool.tile([P, 129], F32, name=f"acc{atag}",
                                          tag=f"acc{atag}")
                        nc.vector.tensor_tensor(out=accN[:, :], in0=acc[:, :],
                                                in1=red[:, :], op=ALU.add)
                        acc = accN
                    post_block(i, num_ps, acc)

            def finish_x(num_ps, acc, brep_sb):
                xs = wpool.tile([P, 129], F32, name="xs", tag="xs")
                nc.vector.tensor_tensor(out=xs[:, :], in0=num_ps[:, :],
                                        in1=acc[:, :], op=ALU.add)
                den = spool.tile([P, 1], F32, name="den", tag="den")
                nc.vector.scalar_tensor_tensor(
                    out=den[:, :], in0=xs[:, 128:129], scalar=1e-30,
                    in1=xs[:, 128:129], op0=ALU.max, op1=ALU.max)
                rec = spool.tile([P, 1], F32, name="rec", tag="rec")
                nc.vector.reciprocal(rec[:, :], den[:, :])
                xp = wpool.tile([P, H], F32, name="xp", tag="xp")
                nc.vector.scalar_tensor_tensor(
                    out=xp[:, :], in0=xs[:, 0:128], scalar=rec[:, :],
                    in1=brep_sb[:, :], op0=ALU.mult, op1=ALU.add)
                xr = wpool.tile([P, H], F32, name="xr", tag="xr")
                nc.scalar.activation(xr[:, :], xp[:, :], AF.Relu)
                xm = wpool.tile([P, H], F32, name="xm", tag="xm")
                nc.vector.tensor_tensor(out=xm[:, :], in0=xp[:, :],
                                        in1=xr[:, :], op=ALU.subtract)
                xe = wpool.tile([P, H], F32, name="xe", tag="xe")
                nc.scalar.activation(xe[:, :], xm[:, :], AF.Exp)
                x2 = wpool.tile([P, H], F32, name="x2", tag="x2")
                nc.vector.scalar_tensor_tensor(
                    out=x2[:, :], in0=xe[:, :], scalar=-1.0, in1=xr[:, :],
                    op0=ALU.add, op1=ALU.add)
                return x2

            def post1(i, num_ps, acc):
                x2 = finish_x(num_ps, acc, b1rep_sb)
                xt_ps = pspool.tile([P, P], F32, name="x2t_ps", tag="tps")
                nc.tensor.transpose(xt_ps[:, :], x2[:, :], ident_sb[:, :])
                x2t = wpool.tile([P, P], F32, name="x2t", tag="x2t")
                nc.scalar.activation(x2t[:, :], xt_ps[:, :], AF.Copy)
                hs_ps = pspool.tile([P, WCOL], F32, name="hs2_ps", tag="hsps")
                nc.tensor.matmul(hs_ps[:, :], lhsT=x2t[:, :], rhs=W2e[:, :],
                                 start=True, stop=True)
                store_hs(i, hs_ps, hs2_own, hs2_sh, s2src, s2dst)

            edge_phase(hs1_f, hs1_own, s1src, s1dst, post1)

            if analysis:
                nc.sync.dma_start(hs2_f[0:SH, :], hs2_sh[:, :])
            else:
                nc.gpsimd.collective_compute(
                    "AllGather", ALU.bypass, groups,
                    ins=[hs2_sh[:, :]], outs=[hs2_f[:, :]])

            pool_ps = apool.tile([G, O], F32, name="pool_ps", tag="poolps")

            def post2(i, num_ps, acc):
                x3 = finish_x(num_ps, acc, b2rep_sb)
                xt_ps = pspool.tile([P, P], F32, name="x3t_ps", tag="tps")
                nc.tensor.transpose(xt_ps[:, :], x3[:, :], ident_sb[:, :])
                x3t = wpool.tile([P, P], F32, name="x3t", tag="x2t")
                nc.scalar.activation(x3t[:, :], xt_ps[:, :], AF.Copy)
                y_ps = pspool.tile([P, O], F32, name="y_ps", tag="hsps")
                nc.tensor.matmul(y_ps[:, :], lhsT=x3t[:, :],
                                 rhs=Wlin_sb[:, :], start=True, stop=True)
                y_sb = wpool.tile([P, O], F32, name="y_sb", tag="y_sb")
                nc.vector.tensor_tensor(out=y_sb[:, :], in0=y_ps[:, :],
                                        in1=blinrep_sb[:, :], op=ALU.add)
                Mg = wpool.tile([P, G], F32, name="Mg", tag="Mg")
                nc.vector.scalar_tensor_tensor(
                    out=Mg[:, :], in0=iota_sb[:, 0:G],
                    scalar=brel_sb[:, i:i + 1], in1=zeros64[:, :],
                    op0=ALU.is_equal, op1=ALU.max)
                nc.tensor.matmul(pool_ps[:, :], lhsT=Mg[:, :], rhs=y_sb[:, :],
                                 start=(i == 0), stop=(i == NBPC - 1))

            edge_phase(hs2_f, hs2_own, s2src, s2dst, post2)

            # ---------------- final reduce ----------------
            pool_sb = spool.tile([G, O], F32, name="pool_sb", tag="pool_sb")
            nc.scalar.activation(pool_sb[:, :], pool_ps[:, :], AF.Copy)
            nc.sync.dma_start(pool_in[:, :], pool_sb[:, :])
            if analysis:
                nc.sync.dma_start(pool_out[:, :], pool_in[:, :])
            else:
                nc.gpsimd.collective_compute(
                    "AllReduce", ALU.add, groups,
                    ins=[pool_in[:, :]], outs=[pool_out[:, :]])
            red_sb = spool.tile([G, O], F32, name="red_sb", tag="red_sb")
            nc.sync.dma_start(red_sb[:, :], pool_out[:, :])
            fin_sb = spool.tile([G, O], F32, name="fin_sb", tag="fin_sb")
            nc.vector.tensor_scalar(out=fin_sb[:, :], in0=red_sb[:, :],
                                    scalar1=cinv_sb[:, :], scalar2=None,
                                    op0=ALU.mult)
            nc.sync.dma_start(t_out.ap(), fin_sb[:, :])

    nc.compile()
    nc.m = get_hw_module(nc.m)
    return nc


_CACHE = {}


def _get_nc(meta):
    key = (meta["CHT"], meta["CH"])
    if key not in _CACHE:
        _CACHE[key] = _build(meta)
    return _CACHE[key]


def run(inputs, trace=False, **kw):
    meta, percore, consts = _prep(inputs)
    nc = _get_nc(meta)
    in_maps = []
    for c in range(NCORES):
        m = dict(consts)
        for k, v in percore.items():
            m[k] = np.ascontiguousarray(v[c])
        in_maps.append(m)
    res = bass_utils.run_bass_kernel_spmd(
        nc, in_maps, core_ids=list(range(NCORES)), trace=trace, **kw)
    return res


def kernel(**inputs):
    res = run(inputs, trace=False)
    return res.results[0]["out"]
